# revision 22
# baseline (speedup 1.0000x reference)
"""Trainium2 Bass kernel for nn_FilmLayerNetwork.

Sharding: one NeuronCore per SMAB processor (NPROC = 8 = n_cores).
Each core computes its processor's full 512-map output slice.

Performance notes (from NTFF traces):
- fp32 matmuls run as LOW/HIGH pass pairs with a ~300ns-per-instruction
  floor, so matmul COUNT dominates PE time. Stage 0 computes [Qk|Kk|Vv]
  with 4 matmuls (fused 288-wide rhs, P as the 8-col stationary) and the
  per-head transposed views are recovered with one DVE 32x32 block
  transpose; independent matmul groups (alphaT, qT) are hand-interleaved
  into the serial attention chain's PE gaps (the PE runs its queue in
  order, so emission order is placement).
- HWDGE services all queued transfers serially in global issue order, so
  all input DMAs go on ONE queue in consumption order, with w1/wa split
  into chunks so dependent matmuls start as each chunk lands.
- de = (De*gate).sum(-1) runs on the vector engine; the FiLM tail runs
  in a (128,4) layout (128-partition vector ops are ~150ns vs ~600-3300ns
  for 1-partition ops); sigmoid is computed as 1/(1+exp(-x)) to keep a
  single scalar-engine activation table resident (a swap costs 1.3us).
- All matmuls sit at PE tile position (0,0): no partition-offset operands
  (cross-tile PSUM-bank hazards) and no PE transposes.
"""

import numpy as np

NM, ZG, HID, SEQ = 512, 512, 96, 8
H1, H2, NPROC, NB = 3, 16, 8, 2
SCL = float(1.0 / np.sqrt(96.0))

# b96 column layout
C_F1, C_WQ2, C_WK2, C_WV2, C_F2, C_WO, C_E, C_BQR = (
    0, 96, 192, 288, 384, 480, 992, 1008)
B96_COLS = 1009
# t128 column layout: baT | regsT | offsT | De_pm | gate128
C_BA, C_RG, C_OF, C_DE, C_G = 0, 4, 8, 9, 41
T128_COLS = 49

_CACHE = {}


def _build_nc():
    import concourse.bass as bass
    import concourse.bacc as bacc
    import concourse.tile as tile
    import concourse.mybir as mybir

    f32 = mybir.dt.float32
    AX = mybir.AxisListType
    ALU = mybir.AluOpType
    ACT = mybir.ActivationFunctionType

    nc = bacc.Bacc("TRN2", target_bir_lowering=False, debug=False, num_devices=NPROC)

    d_px = nc.dram_tensor("px", [128, 36], f32, kind="ExternalInput").ap()
    d_w1 = nc.dram_tensor("w1", [128, 1152], f32, kind="ExternalInput").ap()
    d_b96 = nc.dram_tensor("b96", [96, B96_COLS], f32, kind="ExternalInput").ap()
    d_wqr = nc.dram_tensor("wqr", [128, 384], f32, kind="ExternalInput").ap()
    d_b16 = nc.dram_tensor("b16", [16, 96], f32, kind="ExternalInput").ap()
    d_t128 = nc.dram_tensor("t128", [128, T128_COLS], f32, kind="ExternalInput").ap()
    d_wa = nc.dram_tensor("wa", [128, 2048], f32, kind="ExternalInput").ap()
    d_out = nc.dram_tensor("out", [128, 4], f32, kind="ExternalOutput").ap()

    with tile.TileContext(nc) as tc, \
         tc.tile_pool(name="sb", bufs=1) as sb, \
         tc.tile_pool(name="ps", bufs=8, space="PSUM") as ps:

        def sbt(shape, tag, dt=f32):
            return sb.tile(shape, dt, tag=tag, name=tag)

        def pst(shape, tag):
            return ps.tile(shape, f32, tag="ps_shared", name=tag)

        # ---- input DMAs: one HWDGE queue, consumption order, big tensors
        # chunked so consumers start early ----
        sb_px = sbt([128, 36], "sb_px")
        nc.scalar.dma_start(out=sb_px[:], in_=d_px[:])
        sb_w1k = []
        for k in range(4):
            t = sbt([128, 288], f"sb_w1k{k}")
            nc.scalar.dma_start(out=t[:], in_=d_w1[:, 288 * k:288 * k + 288])
            sb_w1k.append(t)
        # wa as ONE transfer: 8KB rows stream at ~424 GB/s vs ~185 GB/s for
        # 2KB rows (per-packet overhead dominates small rows). It lands
        # right after w1 so the 16 alphaT matmuls can fill PE gaps in the
        # serial attention chain.
        sb_wa = sbt([128, 2048], "sb_wa")
        nc.scalar.dma_start(out=sb_wa[:], in_=d_wa[:])
        sb_wqr = sbt([128, 384], "sb_wqr")
        nc.scalar.dma_start(out=sb_wqr[:], in_=d_wqr[:])
        sb_t = sbt([128, T128_COLS], "sb_t")
        nc.scalar.dma_start(out=sb_t[:], in_=d_t128[:])
        sb_16 = sbt([16, 96], "sb_16")
        nc.scalar.dma_start(out=sb_16[:], in_=d_b16[:])
        sb_96 = sbt([96, B96_COLS], "sb_96")
        nc.scalar.dma_start(out=sb_96[:], in_=d_b96[:])

        P_blk = lambda k: sb_px[:, 8 * k:8 * k + 8]
        xT_blk = lambda k: sb_px[:, 32 + k:33 + k]

        # ---- stage 0: [Qk | Kk | Vv] (8, 288) in 4 fused matmuls ----
        ps_qkv = pst([8, 288], "ps_qkv")
        for k in range(4):
            nc.tensor.matmul(ps_qkv[:], P_blk(k), sb_w1k[k][:],
                             start=(k == 0), stop=(k == 3))

        sb_qkv = sbt([8, 288], "sb_qkv")
        nc.scalar.copy(sb_qkv[:], ps_qkv[:])
        Qk = sb_qkv[:, 0:96]
        Kk = sb_qkv[:, 96:192]
        Vv = sb_qkv[:, 192:288]

        # per-head transposed views of Qk and Kk via one DVE block transpose
        sb_t32 = sbt([32, 192], "sb_t32")
        nc.vector.memset(sb_t32[:], 0.0)
        nc.scalar.copy(sb_t32[0:8, 0:96], Qk)
        nc.scalar.copy(sb_t32[0:8, 96:192], Kk)
        sb_tT = sbt([32, 192], "sb_tT")
        nc.vector.transpose(sb_tT[:], sb_t32[:])

        def QkT_h(h):
            return sb_tT[0:32, 32 * h:32 * h + 8]

        def KkT_h(h):
            return sb_tT[0:32, 96 + 32 * h:96 + 32 * h + 8]

        # MHA1 scores, per head, all at PE tile position (0,0)
        ps_s = pst([8, 24], "ps_s")
        for h in range(3):
            nc.tensor.matmul(ps_s[:, 8 * h:8 * h + 8], QkT_h(h), KkT_h(h))

        # QkT (96,8) contiguous for the attention residual
        sb_qkT = sbt([96, 8], "sb_qkT")
        for h in range(3):
            nc.scalar.copy(sb_qkT[32 * h:32 * h + 32, :], QkT_h(h))

        # softmax (magnitudes are small; max-subtraction unnecessary),
        # normalized A written directly into the 32x32-block layout
        sb_a32 = sbt([32, 96], "sb_a32")
        nc.vector.memset(sb_a32[:], 0.0)
        a32v = sb_a32[0:8, :].rearrange("p (h x) -> p h x", h=3)[:, :, 0:8]
        nc.scalar.activation(a32v, ps_s[:].rearrange("p (h x) -> p h x", h=3),
                             ACT.Exp, scale=SCL)
        sb_sums = sbt([8, 3], "sb_sums")
        nc.vector.tensor_reduce(sb_sums[:], a32v, AX.X, ALU.add)
        sb_rec = sbt([8, 3], "sb_rec")
        nc.vector.reciprocal(sb_rec[:], sb_sums[:])
        rec_ap = sb_rec[:]
        rec_bc = bass.AP(tensor=rec_ap.tensor, offset=rec_ap.offset,
                         ap=[rec_ap.ap[0], rec_ap.ap[1], [0, 8]])
        nc.vector.tensor_tensor(a32v, a32v, rec_bc, ALU.mult)
        sb_aT32 = sbt([32, 96], "sb_aT32")
        nc.vector.transpose(sb_aT32[:], sb_a32[:])

        def A_T(h):
            return sb_aT32[0:8, 32 * h:32 * h + 8]

        sb_vm = sbt([8, 288], "sb_vm")
        nc.vector.memset(sb_vm[:], 0.0)
        for h in range(3):
            nc.scalar.copy(sb_vm[:, 128 * h:128 * h + 32],
                           Vv[:, 32 * h:32 * h + 32])

        # alphaT (128,4): 16 (k,m) chunk matmuls, interleaved into the
        # chain's PE gaps; each group m only needs wa chunk m
        ps_al = pst([128, 4], "ps_al")

        def alpha_mms(ms):
            for m in ms:
                for k in range(4):
                    nc.tensor.matmul(
                        ps_al[:, m:m + 1],
                        sb_wa[:, 512 * k + 128 * m:512 * k + 128 * m + 128],
                        xT_blk(k), start=(k == 0), stop=(k == 3))

        alpha_mms([0])

        # qT: contraction chunks over Wqr (wqr lands right after wa)
        ps_qT = pst([96, 1], "ps_qT")
        for k in range(4):
            nc.tensor.matmul(ps_qT[:], sb_wqr[:, 96 * k:96 * k + 96], xT_blk(k),
                             start=(k == 0), stop=(k == 3))

        # O^T = sum_h maskedV_h^T @ A_T_h at tile position (0,0)
        ps_oT = pst([96, 8], "ps_oT")
        for h in range(3):
            nc.tensor.matmul(ps_oT[:], sb_vm[:, 96 * h:96 * h + 96], A_T(h),
                             start=(h == 0), stop=(h == 2))
        sb_hT = sbt([96, 8], "sb_hT")
        nc.vector.tensor_add(sb_hT[:], ps_oT[:], sb_qkT[:])

        alpha_mms([1])

        # ---- fc1 residual (transposed orientation) ----
        ps_rT = pst([96, 8], "ps_rT")
        nc.tensor.matmul(ps_rT[:], sb_96[:, C_F1:C_F1 + 96], sb_hT[:])
        sb_rT = sbt([96, 8], "sb_rT")
        nc.scalar.activation(sb_rT[:], ps_rT[:], ACT.Relu)
        sb_h2T = sbt([96, 8], "sb_h2T")
        nc.vector.tensor_add(sb_h2T[:], sb_hT[:], sb_rT[:])

        alpha_mms([2])

        # de on the vector engine: (128,4,8) * gate -> reduce over SEQ
        sb_de = sbt([128, 4], "sb_de")
        sb_dp = sbt([128, 32], "sb_dp")
        de_v = sb_t[:, C_DE:C_DE + 32].rearrange("p (m s) -> p m s", m=4)
        g_ap = sb_t[:, C_G:C_G + 8]
        g_bc = bass.AP(tensor=g_ap.tensor, offset=g_ap.offset,
                       ap=[g_ap.ap[0], [0, 4], g_ap.ap[1]])
        nc.vector.tensor_tensor(sb_dp[:].rearrange("p (m s) -> p m s", m=4),
                                de_v, g_bc, ALU.mult)
        nc.vector.tensor_reduce(sb_de[:],
                                sb_dp[:].rearrange("p (m s) -> p m s", m=4),
                                AX.X, ALU.add)

        # ---- stage 2: task query attends to the set (16 heads, d=6) ----
        sb_qT = sbt([96, 1], "sb_qT")
        nc.scalar.activation(sb_qT[:], ps_qT[:], ACT.Relu,
                             bias=sb_96[:, C_BQR:C_BQR + 1])
        ps_k2T = pst([96, 8], "ps_k2T")
        nc.tensor.matmul(ps_k2T[:], sb_96[:, C_WK2:C_WK2 + 96], sb_h2T[:])
        ps_v2T = pst([96, 8], "ps_v2T")
        nc.tensor.matmul(ps_v2T[:], sb_96[:, C_WV2:C_WV2 + 96], sb_h2T[:])
        ps_qqT = pst([96, 1], "ps_qqT")
        nc.tensor.matmul(ps_qqT[:], sb_96[:, C_WQ2:C_WQ2 + 96], sb_qT[:])
        sb_qqT = sbt([96, 1], "sb_qqT")
        nc.scalar.copy(sb_qqT[:], ps_qqT[:])
        sb_v2T = sbt([96, 8], "sb_v2T")
        nc.scalar.copy(sb_v2T[:], ps_v2T[:])
        sb_tmp = sbt([96, 8], "sb_tmp")
        nc.scalar.mul(sb_tmp[:], ps_k2T[:], sb_qqT[:])

        alpha_mms([3])

        ps_s2 = pst([16, 8], "ps_s2")
        nc.tensor.matmul(ps_s2[:], sb_96[:, C_E:C_E + 16], sb_tmp[:])
        sb_e2 = sbt([16, 8], "sb_e2")
        nc.scalar.activation(sb_e2[:], ps_s2[:], ACT.Exp, scale=SCL)
        sb_sum2 = sbt([16, 1], "sb_sum2")
        nc.vector.tensor_reduce(sb_sum2[:], sb_e2[:], AX.X, ALU.add)
        sb_rec2 = sbt([16, 1], "sb_rec2")
        nc.vector.reciprocal(sb_rec2[:], sb_sum2[:])
        sb_a2 = sbt([16, 8], "sb_a2")
        nc.scalar.mul(sb_a2[:], sb_e2[:], sb_rec2[:])

        # alpha sigmoid tail: 1/(1+exp(-(z+ba))) in (128,4)
        sb_zb = sbt([128, 4], "sb_zb")
        nc.vector.tensor_add(sb_zb[:], ps_al[:], sb_t[:, C_BA:C_BA + 4])
        sb_en = sbt([128, 4], "sb_en")
        nc.scalar.activation(sb_en[:], sb_zb[:], ACT.Exp, scale=-1.0)
        sb_dn = sbt([128, 4], "sb_dn")
        nc.vector.tensor_scalar_add(sb_dn[:], sb_en[:], 1.0)
        sb_alp = sbt([128, 4], "sb_alp")
        nc.vector.reciprocal(sb_alp[:], sb_dn[:])

        ps_a2e = pst([96, 8], "ps_a2e")
        nc.tensor.matmul(ps_a2e[:], sb_16[:], sb_a2[:])
        sb_scr = sbt([96, 8], "sb_scr")
        nc.vector.tensor_mul(sb_scr[:], ps_a2e[:], sb_v2T[:])
        sb_o2T = sbt([96, 1], "sb_o2T")
        nc.vector.tensor_reduce(sb_o2T[:], sb_scr[:], AX.X, ALU.add)
        sb_ot1 = sbt([96, 1], "sb_ot1")
        nc.vector.tensor_add(sb_ot1[:], sb_o2T[:], sb_qqT[:])
        ps_r2 = pst([96, 1], "ps_r2")
        nc.tensor.matmul(ps_r2[:], sb_96[:, C_F2:C_F2 + 96], sb_ot1[:])
        sb_r2 = sbt([96, 1], "sb_r2")
        nc.scalar.activation(sb_r2[:], ps_r2[:], ACT.Relu)
        sb_otf = sbt([96, 1], "sb_otf")
        nc.vector.tensor_add(sb_otf[:], sb_ot1[:], sb_r2[:])

        # ---- tail: transT then FiLM mix, all (128,4) ----
        ps_tr = pst([128, 4], "ps_tr")
        for m in range(4):
            nc.tensor.matmul(ps_tr[:, m:m + 1],
                             sb_96[:, C_WO + 128 * m:C_WO + 128 * m + 128],
                             sb_otf[:])
        sb_d1 = sbt([128, 4], "sb_d1")
        nc.vector.tensor_sub(sb_d1[:], ps_tr[:], sb_de[:])
        sb_d2 = sbt([128, 4], "sb_d2")
        nc.vector.tensor_mul(sb_d2[:], sb_d1[:], sb_alp[:])
        sb_mx = sbt([128, 4], "sb_mx")
        nc.vector.tensor_add(sb_mx[:], sb_d2[:], sb_de[:])
        sb_sc = sbt([128, 4], "sb_sc")
        nc.vector.tensor_mul(sb_sc[:], sb_mx[:], sb_t[:, C_RG:C_RG + 4])
        sb_o = sbt([128, 4], "sb_o")
        nc.vector.tensor_scalar_add(sb_o[:], sb_sc[:], sb_t[:, C_OF:C_OF + 1])

        nc.scalar.dma_start(out=d_out[:], in_=sb_o[:])

    nc.compile()
    return nc


def _to_chunks128(a, cols):
    """(512, cols) -> (128, 4*cols) with column block k = rows [128k, 128k+128)."""
    return np.ascontiguousarray(
        a.reshape(4, 128, cols).transpose(1, 0, 2).reshape(128, 4 * cols),
        dtype=np.float32)


def _pack_inputs(inputs):
    gate = np.asarray(inputs['gate'], np.float32)
    x = np.asarray(inputs['x'], np.float32)
    Wa = np.asarray(inputs['Wa'], np.float32)
    ba = np.asarray(inputs['ba'], np.float32)
    Wqr = np.asarray(inputs['Wqr'], np.float32)
    bqr = np.asarray(inputs['bqr'], np.float32)
    P = np.asarray(inputs['P'], np.float32)
    De = np.asarray(inputs['De'], np.float32)
    regs = np.asarray(inputs['regs'], np.float32)

    wa_p = _to_chunks128(Wa, 512)
    wqr_p = _to_chunks128(Wqr, 96)
    xT4 = np.ascontiguousarray(x.reshape(4, 128).T, dtype=np.float32)
    baT4 = np.ascontiguousarray(ba.reshape(4, 128).T, dtype=np.float32)
    g128 = np.ascontiguousarray(np.tile(gate.reshape(1, 8), (128, 1)))

    E = np.zeros((96, 16), np.float32)
    E[np.arange(96), np.arange(96) // 6] = 1.0
    b16 = np.ascontiguousarray(E.T)

    in_maps = []
    for i in range(NPROC):
        b, t = i // 4, i % 4
        px = np.concatenate([_to_chunks128(P[b, t], 8), xT4], axis=1)
        wq1 = np.asarray(inputs['Wq1'], np.float32)[i]
        wk1 = np.asarray(inputs['Wk1'], np.float32)[i]
        wv1 = np.asarray(inputs['Wv1'], np.float32)[i]
        # w1 chunk-major: block k = [wq1_k | wk1_k | wv1_k], each (128, 96)
        w1 = np.concatenate(
            [np.concatenate([wq1[128 * k:128 * k + 128],
                             wk1[128 * k:128 * k + 128],
                             wv1[128 * k:128 * k + 128]], axis=1)
             for k in range(4)], axis=1)
        b96 = np.concatenate([
            np.asarray(inputs['fc1'], np.float32)[i],
            np.asarray(inputs['Wq2'], np.float32)[i],
            np.asarray(inputs['Wk2'], np.float32)[i],
            np.asarray(inputs['Wv2'], np.float32)[i],
            np.asarray(inputs['fc2'], np.float32)[i],
            np.asarray(inputs['Wo'], np.float32)[i],
            E,
            bqr.reshape(96, 1),
        ], axis=1)
        offs = 1.0 if t in (0, 2) else 0.0
        t128 = np.concatenate([
            baT4,
            np.ascontiguousarray(regs[b, t].reshape(4, 128).T),
            np.full((128, 1), offs, np.float32),
            _to_chunks128(De[b, t], 8),
            g128,
        ], axis=1)
        in_maps.append({
            'px': np.ascontiguousarray(px),
            'w1': np.ascontiguousarray(w1),
            'b96': np.ascontiguousarray(b96),
            'wqr': wqr_p,
            'b16': b16,
            't128': np.ascontiguousarray(t128),
            'wa': wa_p,
        })
    return in_maps


def _run(inputs, trace=False):
    from concourse.bass_utils import run_bass_kernel_spmd
    if 'nc' not in _CACHE:
        _CACHE['nc'] = _build_nc()
    nc = _CACHE['nc']
    in_maps = _pack_inputs(inputs)
    res = run_bass_kernel_spmd(nc, in_maps, list(range(NPROC)), trace=trace)
    out = np.zeros((NB, 4, NM), np.float32)
    for i in range(NPROC):
        out[i // 4, i % 4] = np.asarray(res.results[i]['out']).T.reshape(NM)
    return out, res


def kernel(**inputs):
    out, _ = _run(inputs, trace=False)
    return out


# revision 23
# speedup vs baseline: 1.0107x; 1.0107x over previous
"""Trainium2 Bass kernel for nn_FilmLayerNetwork.

Sharding: one NeuronCore per SMAB processor (NPROC = 8 = n_cores).
Each core computes its processor's full 512-map output slice.

Performance notes (from NTFF traces):
- fp32 matmuls run as LOW/HIGH pass pairs, so matmul count dominates PE
  time; independent matmul groups (alphaT, qT) are hand-interleaved into
  the serial attention chain's PE gaps (the PE executes its queue in
  order, so emission order is placement).
- de = (De*gate).sum(-1) runs on the vector engine (broadcast multiply +
  reduce) instead of 4 matmuls.
- The FiLM tail runs in a (128,4) layout: 128-partition vector ops are
  ~150ns vs ~600-3300ns for 1-partition ops.
- sigmoid is computed as 1/(1+exp(-x)): keeps a single scalar-engine
  activation table resident (a mid-kernel table swap costs 1.3us).
- MHA1 scores use one masked-rhs matmul at PE tile position (0,0);
  partition-offset (array-tiled) matmuls and PE transposes are avoided.
- Input DMAs are split across the two HWDGE queues (scalar + sync),
  ordered by first use. gpsimd SWDGE is avoided (6.9us drain).
"""

import numpy as np

NM, ZG, HID, SEQ = 512, 512, 96, 8
H1, H2, NPROC, NB = 3, 16, 8, 2
SCL = float(1.0 / np.sqrt(96.0))

# b96 column layout
C_F1, C_WQ2, C_WK2, C_WV2, C_F2, C_WO, C_E, C_BQR = (
    0, 96, 192, 288, 384, 480, 992, 1008)
B96_COLS = 1009
# t128 column layout: baT | regsT | offsT | De_pm | gate128
C_BA, C_RG, C_OF, C_DE, C_G = 0, 4, 8, 9, 41
T128_COLS = 49

_CACHE = {}


def _build_nc():
    import concourse.bass as bass
    import concourse.bacc as bacc
    import concourse.tile as tile
    import concourse.mybir as mybir

    f32 = mybir.dt.float32
    AX = mybir.AxisListType
    ALU = mybir.AluOpType
    ACT = mybir.ActivationFunctionType

    nc = bacc.Bacc("TRN2", target_bir_lowering=False, debug=False, num_devices=NPROC)

    d_px = nc.dram_tensor("px", [128, 36], f32, kind="ExternalInput").ap()
    d_w1 = nc.dram_tensor("w1", [128, 1152], f32, kind="ExternalInput").ap()
    d_b96 = nc.dram_tensor("b96", [96, B96_COLS], f32, kind="ExternalInput").ap()
    d_wqr = nc.dram_tensor("wqr", [128, 384], f32, kind="ExternalInput").ap()
    d_b16 = nc.dram_tensor("b16", [16, 96], f32, kind="ExternalInput").ap()
    d_t128 = nc.dram_tensor("t128", [128, T128_COLS], f32, kind="ExternalInput").ap()
    d_wa = nc.dram_tensor("wa", [128, 2048], f32, kind="ExternalInput").ap()
    d_out = nc.dram_tensor("out", [128, 4], f32, kind="ExternalOutput").ap()
    d_warm = nc.dram_tensor("warm", [1, 1], f32, kind="ExternalOutput").ap()

    with tile.TileContext(nc) as tc, \
         tc.tile_pool(name="sb", bufs=1) as sb, \
         tc.tile_pool(name="ps", bufs=8, space="PSUM") as ps:

        def sbt(shape, tag):
            return sb.tile(shape, f32, tag=tag, name=tag)

        def pst(shape, tag):
            return ps.tile(shape, f32, tag="ps_shared", name=tag)

        # ---- input DMAs: ONE HWDGE queue in priority order (the HWDGE
        # services all queued transfers serially in global issue order, so
        # a single queue with deliberate ordering beats two racing queues)
        sb_px = sbt([128, 36], "sb_px")
        nc.scalar.dma_start(out=sb_px[:], in_=d_px[:])
        sb_w1 = sbt([128, 1152], "sb_w1")
        nc.scalar.dma_start(out=sb_w1[:], in_=d_w1[:])
        sb_wqr = sbt([128, 384], "sb_wqr")
        nc.scalar.dma_start(out=sb_wqr[:], in_=d_wqr[:])
        sb_wa = sbt([128, 2048], "sb_wa")
        nc.scalar.dma_start(out=sb_wa[:], in_=d_wa[:])
        sb_96 = sbt([96, B96_COLS], "sb_96")
        nc.scalar.dma_start(out=sb_96[:], in_=d_b96[:])
        sb_t = sbt([128, T128_COLS], "sb_t")
        nc.scalar.dma_start(out=sb_t[:], in_=d_t128[:])
        sb_16 = sbt([16, 96], "sb_16")
        nc.scalar.dma_start(out=sb_16[:], in_=d_b16[:])

        P_blk = lambda k: sb_px[:, 8 * k:8 * k + 8]
        xT_blk = lambda k: sb_px[:, 32 + k:33 + k]

        # ---- PE warm-up: ~4us of back-to-back dummy matmuls while the
        # input DMAs stream in. The PE_HAM clock gate defaults to 1.2 GHz
        # and only releases to 2.4 GHz after ~3.4us of sustained activity;
        # without this, every real matmul below runs at half clock.
        sb_wrm = sbt([128, 512], "sb_wrm")
        nc.vector.memset(sb_wrm[:], 0.0)
        ps_wrm = pst([128, 512], "ps_wrm")
        for _ in range(6):
            nc.tensor.matmul(ps_wrm[:], sb_wrm[:, 0:128], sb_wrm[:])
        sb_wout = sbt([1, 1], "sb_wout")
        nc.vector.tensor_reduce(sb_wout[:], ps_wrm[0:1, 0:4], AX.X, ALU.add)
        nc.sync.dma_start(out=d_warm[:], in_=sb_wout[:])

        # ---- stage 0: QkT / KkT (96,8), Vv (8,96) ----
        ps_qkT = pst([96, 8], "ps_qkT")
        ps_kkT = pst([96, 8], "ps_kkT")
        ps_vv = pst([8, 96], "ps_vv")
        for k in range(4):
            s, e = k == 0, k == 3
            wq1 = sb_w1[:, 96 * k:96 * k + 96]
            wk1 = sb_w1[:, 384 + 96 * k:384 + 96 * k + 96]
            wv1 = sb_w1[:, 768 + 96 * k:768 + 96 * k + 96]
            nc.tensor.matmul(ps_kkT[:], wk1, P_blk(k), start=s, stop=e)
            nc.tensor.matmul(ps_qkT[:], wq1, P_blk(k), start=s, stop=e)
            nc.tensor.matmul(ps_vv[:], P_blk(k), wv1, start=s, stop=e)

        # psum -> sbuf copies + masked KkT build (scalar engine)
        sb_qkT = sbt([96, 8], "sb_qkT")
        nc.scalar.copy(sb_qkT[:], ps_qkT[:])
        sb_kkTm = sbt([96, 24], "sb_kkTm")
        nc.vector.memset(sb_kkTm[:], 0.0)
        for h in range(3):
            nc.scalar.copy(sb_kkTm[32 * h:32 * h + 32, 8 * h:8 * h + 8],
                           ps_kkT[32 * h:32 * h + 32, :])
        sb_vv = sbt([8, 96], "sb_vv")
        nc.scalar.copy(sb_vv[:], ps_vv[:])

        # MHA1 scores: one masked-rhs matmul, S (8, 3x8)
        ps_s = pst([8, 24], "ps_s")
        nc.tensor.matmul(ps_s[:], sb_qkT[:], sb_kkTm[:])

        # qT: emitted after S so the attention chain starts ASAP; wqr is
        # also one of the later DMAs to land
        ps_qT = pst([96, 1], "ps_qT")
        for k in range(4):
            nc.tensor.matmul(ps_qT[:], sb_wqr[:, 96 * k:96 * k + 96], xT_blk(k),
                             start=(k == 0), stop=(k == 3))

        # softmax (magnitudes are small; max-subtraction unnecessary),
        # normalized A written directly into the 32x32-block layout
        sb_a32 = sbt([32, 96], "sb_a32")
        nc.vector.memset(sb_a32[:], 0.0)
        a32v = sb_a32[0:8, :].rearrange("p (h x) -> p h x", h=3)[:, :, 0:8]
        nc.scalar.activation(a32v, ps_s[:].rearrange("p (h x) -> p h x", h=3),
                             ACT.Exp, scale=SCL)
        sb_sums = sbt([8, 3], "sb_sums")
        nc.vector.tensor_reduce(sb_sums[:], a32v, AX.X, ALU.add)
        sb_rec = sbt([8, 3], "sb_rec")
        nc.vector.reciprocal(sb_rec[:], sb_sums[:])
        rec_ap = sb_rec[:]
        rec_bc = bass.AP(tensor=rec_ap.tensor, offset=rec_ap.offset,
                         ap=[rec_ap.ap[0], rec_ap.ap[1], [0, 8]])
        nc.vector.tensor_tensor(a32v, a32v, rec_bc, ALU.mult)
        sb_aT32 = sbt([32, 96], "sb_aT32")
        nc.vector.transpose(sb_aT32[:], sb_a32[:])

        def A_T(h):
            return sb_aT32[0:8, 32 * h:32 * h + 8]

        sb_vm = sbt([8, 288], "sb_vm")
        nc.vector.memset(sb_vm[:], 0.0)
        for h in range(3):
            nc.scalar.copy(sb_vm[:, 128 * h:128 * h + 32],
                           sb_vv[:, 32 * h:32 * h + 32])

        # alphaT part 1: first 4 chunks, overlapping the softmax handoffs
        ps_al = pst([128, 4], "ps_al")

        def alpha_mms(ms):
            for m in ms:
                for k in range(4):
                    nc.tensor.matmul(
                        ps_al[:, m:m + 1],
                        sb_wa[:, 512 * k + 128 * m:512 * k + 128 * m + 128],
                        xT_blk(k), start=(k == 0), stop=(k == 3))

        alpha_mms([0])

        # O^T = sum_h maskedV_h^T @ A_T_h at tile position (0,0)
        ps_oT = pst([96, 8], "ps_oT")
        for h in range(3):
            nc.tensor.matmul(ps_oT[:], sb_vm[:, 96 * h:96 * h + 96], A_T(h),
                             start=(h == 0), stop=(h == 2))
        sb_hT = sbt([96, 8], "sb_hT")
        nc.vector.tensor_add(sb_hT[:], ps_oT[:], sb_qkT[:])

        alpha_mms([1])

        # ---- fc1 residual (transposed orientation) ----
        ps_rT = pst([96, 8], "ps_rT")
        nc.tensor.matmul(ps_rT[:], sb_96[:, C_F1:C_F1 + 96], sb_hT[:])
        sb_rT = sbt([96, 8], "sb_rT")
        nc.scalar.activation(sb_rT[:], ps_rT[:], ACT.Relu)
        sb_h2T = sbt([96, 8], "sb_h2T")
        nc.vector.tensor_add(sb_h2T[:], sb_hT[:], sb_rT[:])

        alpha_mms([2])

        # de on the vector engine: (128,4,8) * gate -> reduce over SEQ
        sb_de = sbt([128, 4], "sb_de")
        sb_dp = sbt([128, 32], "sb_dp")
        de_v = sb_t[:, C_DE:C_DE + 32].rearrange("p (m s) -> p m s", m=4)
        g_ap = sb_t[:, C_G:C_G + 8]
        g_bc = bass.AP(tensor=g_ap.tensor, offset=g_ap.offset,
                       ap=[g_ap.ap[0], [0, 4], g_ap.ap[1]])
        nc.vector.tensor_tensor(sb_dp[:].rearrange("p (m s) -> p m s", m=4),
                                de_v, g_bc, ALU.mult)
        nc.vector.tensor_reduce(sb_de[:],
                                sb_dp[:].rearrange("p (m s) -> p m s", m=4),
                                AX.X, ALU.add)

        # ---- stage 2: task query attends to the set (16 heads, d=6) ----
        sb_qT = sbt([96, 1], "sb_qT")
        nc.scalar.activation(sb_qT[:], ps_qT[:], ACT.Relu,
                             bias=sb_96[:, C_BQR:C_BQR + 1])
        ps_k2T = pst([96, 8], "ps_k2T")
        nc.tensor.matmul(ps_k2T[:], sb_96[:, C_WK2:C_WK2 + 96], sb_h2T[:])
        ps_v2T = pst([96, 8], "ps_v2T")
        nc.tensor.matmul(ps_v2T[:], sb_96[:, C_WV2:C_WV2 + 96], sb_h2T[:])
        ps_qqT = pst([96, 1], "ps_qqT")
        nc.tensor.matmul(ps_qqT[:], sb_96[:, C_WQ2:C_WQ2 + 96], sb_qT[:])
        sb_qqT = sbt([96, 1], "sb_qqT")
        nc.scalar.copy(sb_qqT[:], ps_qqT[:])
        sb_v2T = sbt([96, 8], "sb_v2T")
        nc.scalar.copy(sb_v2T[:], ps_v2T[:])
        sb_tmp = sbt([96, 8], "sb_tmp")
        nc.scalar.mul(sb_tmp[:], ps_k2T[:], sb_qqT[:])

        alpha_mms([3])

        ps_s2 = pst([16, 8], "ps_s2")
        nc.tensor.matmul(ps_s2[:], sb_96[:, C_E:C_E + 16], sb_tmp[:])
        sb_e2 = sbt([16, 8], "sb_e2")
        nc.scalar.activation(sb_e2[:], ps_s2[:], ACT.Exp, scale=SCL)
        sb_sum2 = sbt([16, 1], "sb_sum2")
        nc.vector.tensor_reduce(sb_sum2[:], sb_e2[:], AX.X, ALU.add)
        sb_rec2 = sbt([16, 1], "sb_rec2")
        nc.vector.reciprocal(sb_rec2[:], sb_sum2[:])
        sb_a2 = sbt([16, 8], "sb_a2")
        nc.scalar.mul(sb_a2[:], sb_e2[:], sb_rec2[:])

        # alpha sigmoid tail: 1/(1+exp(-(z+ba))) in (128,4)
        sb_zb = sbt([128, 4], "sb_zb")
        nc.vector.tensor_add(sb_zb[:], ps_al[:], sb_t[:, C_BA:C_BA + 4])
        sb_en = sbt([128, 4], "sb_en")
        nc.scalar.activation(sb_en[:], sb_zb[:], ACT.Exp, scale=-1.0)
        sb_dn = sbt([128, 4], "sb_dn")
        nc.vector.tensor_scalar_add(sb_dn[:], sb_en[:], 1.0)
        sb_alp = sbt([128, 4], "sb_alp")
        nc.vector.reciprocal(sb_alp[:], sb_dn[:])

        ps_a2e = pst([96, 8], "ps_a2e")
        nc.tensor.matmul(ps_a2e[:], sb_16[:], sb_a2[:])
        sb_scr = sbt([96, 8], "sb_scr")
        nc.vector.tensor_mul(sb_scr[:], ps_a2e[:], sb_v2T[:])
        sb_o2T = sbt([96, 1], "sb_o2T")
        nc.vector.tensor_reduce(sb_o2T[:], sb_scr[:], AX.X, ALU.add)
        sb_ot1 = sbt([96, 1], "sb_ot1")
        nc.vector.tensor_add(sb_ot1[:], sb_o2T[:], sb_qqT[:])
        ps_r2 = pst([96, 1], "ps_r2")
        nc.tensor.matmul(ps_r2[:], sb_96[:, C_F2:C_F2 + 96], sb_ot1[:])
        sb_r2 = sbt([96, 1], "sb_r2")
        nc.scalar.activation(sb_r2[:], ps_r2[:], ACT.Relu)
        sb_otf = sbt([96, 1], "sb_otf")
        nc.vector.tensor_add(sb_otf[:], sb_ot1[:], sb_r2[:])

        # ---- tail: transT then FiLM mix, all (128,4) ----
        ps_tr = pst([128, 4], "ps_tr")
        for m in range(4):
            nc.tensor.matmul(ps_tr[:, m:m + 1],
                             sb_96[:, C_WO + 128 * m:C_WO + 128 * m + 128],
                             sb_otf[:])
        sb_d1 = sbt([128, 4], "sb_d1")
        nc.vector.tensor_sub(sb_d1[:], ps_tr[:], sb_de[:])
        sb_d2 = sbt([128, 4], "sb_d2")
        nc.vector.tensor_mul(sb_d2[:], sb_d1[:], sb_alp[:])
        sb_mx = sbt([128, 4], "sb_mx")
        nc.vector.tensor_add(sb_mx[:], sb_d2[:], sb_de[:])
        sb_sc = sbt([128, 4], "sb_sc")
        nc.vector.tensor_mul(sb_sc[:], sb_mx[:], sb_t[:, C_RG:C_RG + 4])
        sb_o = sbt([128, 4], "sb_o")
        nc.vector.tensor_scalar_add(sb_o[:], sb_sc[:], sb_t[:, C_OF:C_OF + 1])

        nc.scalar.dma_start(out=d_out[:], in_=sb_o[:])

    nc.compile()
    return nc


def _to_chunks128(a, cols):
    """(512, cols) -> (128, 4*cols) with column block k = rows [128k, 128k+128)."""
    return np.ascontiguousarray(
        a.reshape(4, 128, cols).transpose(1, 0, 2).reshape(128, 4 * cols),
        dtype=np.float32)


def _pack_inputs(inputs):
    gate = np.asarray(inputs['gate'], np.float32)
    x = np.asarray(inputs['x'], np.float32)
    Wa = np.asarray(inputs['Wa'], np.float32)
    ba = np.asarray(inputs['ba'], np.float32)
    Wqr = np.asarray(inputs['Wqr'], np.float32)
    bqr = np.asarray(inputs['bqr'], np.float32)
    P = np.asarray(inputs['P'], np.float32)
    De = np.asarray(inputs['De'], np.float32)
    regs = np.asarray(inputs['regs'], np.float32)

    wa_p = _to_chunks128(Wa, 512)
    wqr_p = _to_chunks128(Wqr, 96)
    xT4 = np.ascontiguousarray(x.reshape(4, 128).T, dtype=np.float32)
    baT4 = np.ascontiguousarray(ba.reshape(4, 128).T, dtype=np.float32)
    g128 = np.ascontiguousarray(np.tile(gate.reshape(1, 8), (128, 1)))

    E = np.zeros((96, 16), np.float32)
    E[np.arange(96), np.arange(96) // 6] = 1.0
    b16 = np.ascontiguousarray(E.T)

    in_maps = []
    for i in range(NPROC):
        b, t = i // 4, i % 4
        px = np.concatenate([_to_chunks128(P[b, t], 8), xT4], axis=1)
        w1 = np.concatenate([
            _to_chunks128(np.asarray(inputs['Wq1'], np.float32)[i], 96),
            _to_chunks128(np.asarray(inputs['Wk1'], np.float32)[i], 96),
            _to_chunks128(np.asarray(inputs['Wv1'], np.float32)[i], 96),
        ], axis=1)
        b96 = np.concatenate([
            np.asarray(inputs['fc1'], np.float32)[i],
            np.asarray(inputs['Wq2'], np.float32)[i],
            np.asarray(inputs['Wk2'], np.float32)[i],
            np.asarray(inputs['Wv2'], np.float32)[i],
            np.asarray(inputs['fc2'], np.float32)[i],
            np.asarray(inputs['Wo'], np.float32)[i],
            E,
            bqr.reshape(96, 1),
        ], axis=1)
        offs = 1.0 if t in (0, 2) else 0.0
        t128 = np.concatenate([
            baT4,
            np.ascontiguousarray(regs[b, t].reshape(4, 128).T),
            np.full((128, 1), offs, np.float32),
            _to_chunks128(De[b, t], 8),
            g128,
        ], axis=1)
        in_maps.append({
            'px': np.ascontiguousarray(px),
            'w1': np.ascontiguousarray(w1),
            'b96': np.ascontiguousarray(b96),
            'wqr': wqr_p,
            'b16': b16,
            't128': np.ascontiguousarray(t128),
            'wa': wa_p,
        })
    return in_maps


def _run(inputs, trace=False):
    from concourse.bass_utils import run_bass_kernel_spmd
    if 'nc' not in _CACHE:
        _CACHE['nc'] = _build_nc()
    nc = _CACHE['nc']
    in_maps = _pack_inputs(inputs)
    res = run_bass_kernel_spmd(nc, in_maps, list(range(NPROC)), trace=trace)
    out = np.zeros((NB, 4, NM), np.float32)
    for i in range(NPROC):
        out[i // 4, i % 4] = np.asarray(res.results[i]['out']).T.reshape(NM)
    return out, res


def kernel(**inputs):
    out, _ = _run(inputs, trace=False)
    return out
